# revision 5
# baseline (speedup 1.0000x reference)
"""BipartPool (masked bipartite cluster attention) on 8 Trainium2 NeuronCores.

Problem (from reference):
    B=32 graphs x 512 nodes, E=512, H=8 heads (head_dim 64), RATIO=32 cluster
    queries per graph (identical seeds xcent_base for every graph).
    xcent = MHA(q=tile(xcent_base,B), k=v=x, mask=block-diagonal by graph)

Strategy: data-parallel over graphs, 4 graphs per core. The block-diagonal
mask means each (graph, query-block) only attends to its own 512 nodes, so
scores are computed per graph: [32 q, 512 n] per head instead of the
reference's dense [1024, 16384].

Per-core device pipeline (all matmuls on PE, fp32r except bf16 attention
application):
  1. x shard [2048, 512] loaded naturally, PE-transposed into xT [512, 2048].
  2. KT = (wkT)^T-chunks x xT   -> [512 eout, 512 n] per graph (+bk via ACT).
     V  = xT-chunks x wvT       -> [512 n, 512 eout] per graph (bf16 out).
  3. scores via block-diagonal Q trick: lhsT [128=2 heads' d, 64=2 heads' q]
     one matmul per (graph, head-pair) -> psum [128=(4h x 32q), 512 n].
  4. softmax over free axis without max-subtraction (scores are O(1)):
     ACT Exp with accum_out row-sums, DVE reciprocal + scale (bf16 out).
  5. attn PE-transposed (bf16), AV matmuls produce OT packed in one psum
     bank [128, 512] = [e within chunk, (chunk | graph | q)].
  6. out-projection: OT-chunks x woT + bias outer-product -> [128 q, 512].

Host precomputes (weight-only work): wk/wv/out_proj transposes, the 32x512
query projection Q = xcent_base @ wq.T + bq (identical for every graph),
scaled by 1/sqrt(64) and laid out block-diagonally, and the folded output
bias bo' = Wo @ bv + bo (valid because attention rows sum to 1).
"""

import numpy as np

import concourse.bacc as bacc
import concourse.mybir as mybir
from concourse.tile import TileContext
from concourse.bass_utils import run_bass_kernel_spmd
from concourse.masks import make_identity

F32 = mybir.dt.float32
F32R = mybir.dt.float32r
BF16 = mybir.dt.bfloat16

B, RATIO, E, H = 32, 32, 512, 8
HD = E // H  # 64
NODES_PER_GRAPH = 512
N = B * NODES_PER_GRAPH  # 16384
NCORES = 8
GPC = B // NCORES  # graphs per core = 4
NSH = GPC * NODES_PER_GRAPH  # nodes per core = 2048
NB = NSH // 128  # node blocks per core = 16
EC = E // 128  # feature chunks = 4

_NC_CACHE = {}
LAST_RESULT = None  # test harness reads exec_time_ns from here


def _build_nc():
    nc = bacc.Bacc(None, target_bir_lowering=False)

    xs_d = nc.dram_tensor("xs", [NSH, E], F32, kind="ExternalInput")
    wkT_d = nc.dram_tensor("wkT", [E, E], F32R, kind="ExternalInput")
    wvT_d = nc.dram_tensor("wvT", [E, E], F32R, kind="ExternalInput")
    woT_d = nc.dram_tensor("woT", [E, E], F32R, kind="ExternalInput")
    qtbd_d = nc.dram_tensor("qtbd", [128, 512], F32R, kind="ExternalInput")
    bkT_d = nc.dram_tensor("bkT", [E, 1], F32, kind="ExternalInput")
    ones_d = nc.dram_tensor("ones", [1, 128], F32R, kind="ExternalInput")
    bo_d = nc.dram_tensor("bo", [1, E], F32R, kind="ExternalInput")
    out_d = nc.dram_tensor("out", [GPC * RATIO, E], F32, kind="ExternalOutput")

    with TileContext(nc) as tc:
        with (
            tc.tile_pool(name="const", bufs=1) as const,
            tc.tile_pool(name="xta", bufs=1) as xta,
            tc.tile_pool(name="xn", bufs=6) as xnp,
            tc.tile_pool(name="kt", bufs=6) as ktp,
            tc.tile_pool(name="vt", bufs=8) as vtp,
            tc.tile_pool(name="sm", bufs=4) as smp,
            tc.tile_pool(name="fin", bufs=2) as finp,
            tc.tile_pool(name="psw", bufs=2, space="PSUM") as psw,
            tc.tile_pool(name="pst", bufs=2, space="PSUM") as pst,
            tc.tile_pool(name="psa", bufs=2, space="PSUM") as psa,
            tc.tile_pool(name="pso", bufs=1, space="PSUM") as pso,
        ):
            # ---- constants / weights (scalar-engine HWDGE queue) ----
            ident = const.tile([128, 128], F32, tag="idf")
            make_identity(nc, ident)
            identb = const.tile([128, 128], BF16, tag="idb")
            make_identity(nc, identb)

            qtbd = const.tile([128, 512], F32R, tag="qtbd")
            nc.scalar.dma_start(out=qtbd, in_=qtbd_d[:, :])
            bkTs = []
            for c in range(EC):
                t = const.tile([128, 1], F32, tag=f"bk{c}")
                nc.scalar.dma_start(out=t, in_=bkT_d[c * 128 : (c + 1) * 128, :])
                bkTs.append(t)
            ones_r = const.tile([1, 128], F32R, tag="ones")
            nc.scalar.dma_start(out=ones_r, in_=ones_d[:, :])
            bo_row = const.tile([1, E], F32R, tag="bo")
            nc.scalar.dma_start(out=bo_row, in_=bo_d[:, :])

            wkTs, wvTs, woTs = [], [], []
            for name, dram, lst in (
                ("wk", wkT_d, wkTs),
                ("wv", wvT_d, wvTs),
                ("wo", woT_d, woTs),
            ):
                for c in range(EC):
                    t = const.tile([128, E], F32R, tag=f"{name}{c}")
                    nc.scalar.dma_start(out=t, in_=dram[c * 128 : (c + 1) * 128, :])
                    lst.append(t)

            # xT: [512 e, 2048 n] packed as [128, EC * NSH]
            xTall = xta.tile([128, EC * NSH], F32R, tag="xT")
            xT3 = xTall.rearrange("p (c n) -> p c n", c=EC)

            # OT accumulators, one per head parity (matmul psum outputs
            # must start at partition 0): [64 d, (chunk|graph|q) 512]
            otps = [
                pso.tile([64, 512], F32, tag=f"ot{p}", name=f"otps{p}")
                for p in range(2)
            ]

            def emit_transpose_block(g):
                # load 4 node blocks of graph g and transpose into xT
                for nb in range(4 * g, 4 * g + 4):
                    xn = xnp.tile([128, E], F32, tag="xn")
                    nc.sync.dma_start(
                        out=xn, in_=xs_d[nb * 128 : (nb + 1) * 128, :]
                    )
                    pt = pst.tile([128, 512], F32, tag="xtp")
                    for c in range(EC):
                        nc.tensor.transpose(
                            pt[:, c * 128 : (c + 1) * 128],
                            xn[:, c * 128 : (c + 1) * 128],
                            ident,
                        )
                    # one strided copy: psum block c -> xT[c][:, nb*128:...]
                    nc.scalar.activation(
                        out=xT3[:, :, nb * 128 : (nb + 1) * 128],
                        in_=pt.rearrange("p (c n) -> p c n", c=EC),
                        func=mybir.ActivationFunctionType.Copy,
                    )

            def emit_graph(g):
                ns = g * NODES_PER_GRAPH  # node offset in xT free dim

                # ---- K^T per eout chunk: [128 eout, 512 n] ----
                ktiles = []
                for c in range(EC):
                    kps = psw.tile([128, 512], F32, tag="work")
                    for e in range(EC):
                        nc.tensor.matmul(
                            kps,
                            lhsT=wkTs[e][:, c * 128 : (c + 1) * 128],
                            rhs=xTall[:, e * NSH + ns : e * NSH + ns + 512],
                            start=(e == 0),
                            stop=(e == EC - 1),
                        )
                    kt = ktp.tile([128, 512], F32R, tag="kt")
                    nc.scalar.activation(
                        out=kt,
                        in_=kps,
                        func=mybir.ActivationFunctionType.Identity,
                        bias=bkTs[c],
                    )
                    ktiles.append(kt)

                # ---- V per node chunk: [128 n, 512 eout] (bf16) ----
                vtiles = []
                for c in range(EC):
                    vps = psw.tile([128, 512], F32, tag="work")
                    for e in range(EC):
                        nc.tensor.matmul(
                            vps,
                            lhsT=xTall[
                                :, e * NSH + ns + c * 128 : e * NSH + ns + (c + 1) * 128
                            ],
                            rhs=wvTs[e],
                            start=(e == 0),
                            stop=(e == EC - 1),
                        )
                    vt = vtp.tile([128, 512], BF16, tag="vt")
                    nc.vector.tensor_copy(vt, vps)
                    vtiles.append(vt)

                # ---- scores + softmax + AV, per head-quad q (4 heads) ----
                for q in range(2):
                    sps = psw.tile([128, 512], F32, tag="work")
                    for hp2 in range(2):
                        hp = q * 2 + hp2  # head pair == eout chunk of K
                        nc.tensor.matmul(
                            sps,
                            lhsT=qtbd[:, hp * 128 : (hp + 1) * 128],
                            rhs=ktiles[hp],
                            start=(hp2 == 0),
                            stop=(hp2 == 1),
                        )
                    expt = smp.tile([128, 512], F32, tag="exp")
                    sumt = smp.tile([128, 1], F32, tag="sum")
                    nc.scalar.activation(
                        out=expt,
                        in_=sps,
                        func=mybir.ActivationFunctionType.Exp,
                        accum_out=sumt,
                    )
                    rect = smp.tile([128, 1], F32, tag="rec")
                    nc.vector.reciprocal(out=rect, in_=sumt)
                    attnt = smp.tile([128, 512], BF16, tag="attn")
                    nc.vector.tensor_scalar_mul(attnt, expt, rect)

                    atps = psa.tile([128, 512], BF16, tag="atp")
                    for c in range(EC):
                        nc.tensor.transpose(
                            atps[:, c * 128 : (c + 1) * 128],
                            attnt[:, c * 128 : (c + 1) * 128],
                            identb,
                        )
                    attnT = smp.tile([128, 512], BF16, tag="attnT")
                    nc.vector.tensor_copy(attnT, atps)

                    # AV: accumulate OT blocks [64 d, 32 q] over node chunks
                    for hq in range(4):
                        h = 4 * q + hq
                        pcol = (h // 2) * 128 + g * 32
                        for c in range(EC):
                            nc.tensor.matmul(
                                otps[h % 2][:, pcol : pcol + 32],
                                lhsT=vtiles[c][:, h * HD : (h + 1) * HD],
                                rhs=attnT[:, c * 128 + hq * 32 : c * 128 + (hq + 1) * 32],
                                start=(c == 0),
                                stop=(c == EC - 1),
                            )

            emit_transpose_block(0)
            for g in range(GPC):
                if g + 1 < GPC:
                    emit_transpose_block(g + 1)
                emit_graph(g)

            # ---- output projection ----
            ot_sb = finp.tile([128, 512], F32R, tag="otsb")
            for p in range(2):
                nc.scalar.activation(
                    out=ot_sb[p * 64 : (p + 1) * 64, :],
                    in_=otps[p],
                    func=mybir.ActivationFunctionType.Copy,
                )
            xcps = psw.tile([128, 512], F32, tag="work")
            nc.tensor.matmul(xcps, lhsT=ones_r, rhs=bo_row, start=True, stop=False)
            for c in range(EC):
                nc.tensor.matmul(
                    xcps,
                    lhsT=ot_sb[:, c * 128 : (c + 1) * 128],
                    rhs=woTs[c],
                    start=False,
                    stop=(c == EC - 1),
                )
            xc_sb = finp.tile([128, 512], F32, tag="xc")
            nc.scalar.activation(
                out=xc_sb, in_=xcps, func=mybir.ActivationFunctionType.Copy
            )
            nc.sync.dma_start(out=out_d[:, :], in_=xc_sb)

    nc.finalize()
    return nc


def _host_prep(x, xcent_base, in_proj_w, in_proj_b, out_proj_w, out_proj_b):
    x = np.asarray(x, np.float32)
    ipw = np.asarray(in_proj_w, np.float32)
    ipb = np.asarray(in_proj_b, np.float32)
    wo = np.asarray(out_proj_w, np.float32)
    bo = np.asarray(out_proj_b, np.float32)
    xcb = np.asarray(xcent_base, np.float32)

    wq, wk, wv = ipw[0:E], ipw[E : 2 * E], ipw[2 * E : 3 * E]
    bq, bk, bv = ipb[0:E], ipb[E : 2 * E], ipb[2 * E : 3 * E]

    # query projection: identical for every graph; scale folded in
    qs = (xcb @ wq.T + bq) * np.float32(1.0 / np.sqrt(HD))  # [32, 512]
    qtbd = np.zeros((128, 512), np.float32)
    for hp in range(4):
        for a in range(2):
            h = 2 * hp + a
            m0 = hp * 128 + (hp % 2) * 64 + a * 32
            qtbd[a * 64 : (a + 1) * 64, m0 : m0 + 32] = qs[:, h * HD : (h + 1) * HD].T

    consts = dict(
        wkT=np.ascontiguousarray(wk.T),
        wvT=np.ascontiguousarray(wv.T),
        woT=np.ascontiguousarray(wo.T),
        qtbd=qtbd,
        bkT=np.ascontiguousarray(bk[:, None]),
        ones=np.ones((1, 128), np.float32),
        bo=np.ascontiguousarray((wo @ bv + bo)[None, :]),
    )
    return x, consts


def kernel(
    x,
    edge_index,
    batch,
    xcent_base,
    in_proj_w,
    in_proj_b,
    out_proj_w,
    out_proj_b,
):
    global LAST_RESULT
    x, consts = _host_prep(
        x, xcent_base, in_proj_w, in_proj_b, out_proj_w, out_proj_b
    )

    if "nc" not in _NC_CACHE:
        _NC_CACHE["nc"] = _build_nc()
    nc = _NC_CACHE["nc"]

    in_maps = [
        dict(consts, xs=np.ascontiguousarray(x[k * NSH : (k + 1) * NSH]))
        for k in range(NCORES)
    ]
    res = run_bass_kernel_spmd(nc, in_maps, list(range(NCORES)))
    LAST_RESULT = res

    xcent = np.concatenate(
        [res.results[k]["out"].reshape(GPC, RATIO, E) for k in range(NCORES)], axis=0
    ).astype(np.float32)

    batch = np.asarray(batch)
    batchcent = np.repeat(np.arange(B, dtype=batch.dtype), RATIO)
    return xcent, batchcent


# revision 7
# speedup vs baseline: 1.0554x; 1.0554x over previous
"""BipartPool (masked bipartite cluster attention) on 8 Trainium2 NeuronCores.

Problem (from reference):
    B=32 graphs x 512 nodes, E=512, H=8 heads (head_dim 64), RATIO=32 cluster
    queries per graph (identical seeds xcent_base for every graph).
    xcent = MHA(q=tile(xcent_base,B), k=v=x, mask=block-diagonal by graph)

Strategy: data-parallel over graphs, 4 graphs per core. The block-diagonal
mask means each (graph, query-block) only attends to its own 512 nodes, so
scores are computed per graph: [32 q, 512 n] per head instead of the
reference's dense [1024, 16384].

Per-core device pipeline (all matmuls on PE, fp32r except bf16 attention
application):
  1. x shard [2048, 512] loaded naturally, PE-transposed into xT [512, 2048].
  2. KT = (wkT)^T-chunks x xT   -> [512 eout, 512 n] per graph (+bk via ACT).
     V  = xT-chunks x wvT       -> [512 n, 512 eout] per graph (bf16 out).
  3. scores via block-diagonal Q trick: lhsT [128=2 heads' d, 64=2 heads' q]
     one matmul per (graph, head-pair) -> psum [128=(4h x 32q), 512 n].
  4. softmax over free axis without max-subtraction (scores are O(1)):
     ACT Exp with accum_out row-sums, DVE reciprocal + scale (bf16 out).
  5. attn PE-transposed (bf16), AV matmuls produce OT packed in one psum
     bank [128, 512] = [e within chunk, (chunk | graph | q)].
  6. out-projection: OT-chunks x woT + bias outer-product -> [128 q, 512].

Host precomputes (weight-only work): wk/wv/out_proj transposes, the 32x512
query projection Q = xcent_base @ wq.T + bq (identical for every graph),
scaled by 1/sqrt(64) and laid out block-diagonally, and the folded output
bias bo' = Wo @ bv + bo (valid because attention rows sum to 1).
"""

import numpy as np

import concourse.bacc as bacc
import concourse.mybir as mybir
from concourse.tile import TileContext
from concourse.bass_utils import run_bass_kernel_spmd
from concourse.masks import make_identity

F32 = mybir.dt.float32
F32R = mybir.dt.float32r
BF16 = mybir.dt.bfloat16

B, RATIO, E, H = 32, 32, 512, 8
HD = E // H  # 64
NODES_PER_GRAPH = 512
N = B * NODES_PER_GRAPH  # 16384
NCORES = 8
GPC = B // NCORES  # graphs per core = 4
NSH = GPC * NODES_PER_GRAPH  # nodes per core = 2048
NB = NSH // 128  # node blocks per core = 16
EC = E // 128  # feature chunks = 4

_NC_CACHE = {}
LAST_RESULT = None  # test harness reads exec_time_ns from here


def _build_nc():
    nc = bacc.Bacc(None, target_bir_lowering=False)

    xs_d = nc.dram_tensor("xs", [NSH, E], F32R, kind="ExternalInput")
    wkT_d = nc.dram_tensor("wkT", [E, E], F32R, kind="ExternalInput")
    wvT_d = nc.dram_tensor("wvT", [E, E], F32R, kind="ExternalInput")
    woT_d = nc.dram_tensor("woT", [E, E], F32R, kind="ExternalInput")
    qtbd_d = nc.dram_tensor("qtbd", [128, 512], F32R, kind="ExternalInput")
    bkT_d = nc.dram_tensor("bkT", [128, EC], F32, kind="ExternalInput")
    ident_d = nc.dram_tensor("ident", [128, 128], F32R, kind="ExternalInput")
    ones_d = nc.dram_tensor("ones", [1, 128], F32R, kind="ExternalInput")
    bo_d = nc.dram_tensor("bo", [1, E], F32R, kind="ExternalInput")
    out_d = nc.dram_tensor("out", [GPC * RATIO, E], F32, kind="ExternalOutput")

    with TileContext(nc) as tc:
        with (
            tc.tile_pool(name="const", bufs=1) as const,
            tc.tile_pool(name="xta", bufs=1) as xta,
            tc.tile_pool(name="xn", bufs=6) as xnp,
            tc.tile_pool(name="kt", bufs=6) as ktp,
            tc.tile_pool(name="vt", bufs=8) as vtp,
            tc.tile_pool(name="sm", bufs=4) as smp,
            tc.tile_pool(name="fin", bufs=2) as finp,
            tc.tile_pool(name="psw", bufs=2, space="PSUM") as psw,
            tc.tile_pool(name="pst", bufs=2, space="PSUM") as pst,
            tc.tile_pool(name="psa", bufs=2, space="PSUM") as psa,
            tc.tile_pool(name="pso", bufs=1, space="PSUM") as pso,
        ):
            # ---- constants / weights (scalar-engine HWDGE queue) ----
            ident = const.tile([128, 128], F32R, tag="idf")
            nc.gpsimd.dma_start(out=ident, in_=ident_d[:, :])
            identb = const.tile([128, 128], BF16, tag="idb")
            make_identity(nc, identb)

            qtbd = const.tile([128, 512], F32R, tag="qtbd")
            nc.gpsimd.dma_start(out=qtbd, in_=qtbd_d[:, :])
            bkT4 = const.tile([128, EC], F32, tag="bk")
            nc.gpsimd.dma_start(out=bkT4, in_=bkT_d[:, :])
            bkTs = [bkT4[:, c : c + 1] for c in range(EC)]
            ones_r = const.tile([1, 128], F32R, tag="ones")
            nc.gpsimd.dma_start(out=ones_r, in_=ones_d[:, :])
            bo_row = const.tile([1, E], F32R, tag="bo")
            nc.gpsimd.dma_start(out=bo_row, in_=bo_d[:, :])

            # each weight matrix in ONE big DMA: [512, 512] -> [128, (c, 512)]
            wkTs, wvTs, woTs = [], [], []
            for name, dram, lst in (
                ("wk", wkT_d, wkTs),
                ("wv", wvT_d, wvTs),
                ("wo", woT_d, woTs),
            ):
                t4 = const.tile([128, EC * E], F32R, tag=f"{name}all", name=f"{name}all")
                nc.gpsimd.dma_start(
                    out=t4.rearrange("p (c e) -> p c e", c=EC),
                    in_=dram.ap().rearrange("(c p) e -> p c e", p=128),
                )
                for c in range(EC):
                    lst.append(t4[:, c * E : (c + 1) * E])

            # xT: [512 e, 2048 n] packed as [128, EC * NSH]
            xTall = xta.tile([128, EC * NSH], F32R, tag="xT")
            xT3 = xTall.rearrange("p (c n) -> p c n", c=EC)

            # OT accumulators, one per head parity (matmul psum outputs
            # must start at partition 0): [64 d, (chunk|graph|q) 512]
            otps = [
                pso.tile([64, 512], F32, tag=f"ot{p}", name=f"otps{p}")
                for p in range(2)
            ]

            def emit_transpose_block(g):
                # one 1 MiB DMA for the graph's 512 nodes: [512, 512] ->
                # [128, (r, e)] (4 row-blocks side by side)
                xn = xnp.tile([128, 4 * E], F32R, tag="xn", name="xn")
                src = xs_d.ap()[g * 512 : (g + 1) * 512, :].rearrange(
                    "(r p) e -> p r e", p=128
                )
                nc.sync.dma_start(
                    out=xn.rearrange("p (r e) -> p r e", r=4), in_=src
                )
                for r in range(4):
                    nb = 4 * g + r
                    pt = pst.tile([128, 512], F32R, tag="xtp", name="pt")
                    for c in range(EC):
                        nc.tensor.transpose(
                            pt[:, c * 128 : (c + 1) * 128],
                            xn[:, r * E + c * 128 : r * E + (c + 1) * 128],
                            ident,
                        )
                    # one strided copy: psum block c -> xT[c][:, nb*128:...]
                    # alternate ACT / DVE to balance engine load
                    if nb % 2 == 0:
                        nc.scalar.activation(
                            out=xT3[:, :, nb * 128 : (nb + 1) * 128],
                            in_=pt.rearrange("p (c n) -> p c n", c=EC),
                            func=mybir.ActivationFunctionType.Copy,
                        )
                    else:
                        nc.vector.tensor_copy(
                            xT3[:, :, nb * 128 : (nb + 1) * 128],
                            pt.rearrange("p (c n) -> p c n", c=EC),
                        )

            def emit_graph(g):
                ns = g * NODES_PER_GRAPH  # node offset in xT free dim

                # ---- K^T per eout chunk: [128 eout, 512 n] ----
                ktiles = []
                for c in range(EC):
                    kps = psw.tile([128, 512], F32, tag="work")
                    for e in range(EC):
                        nc.tensor.matmul(
                            kps,
                            lhsT=wkTs[e][:, c * 128 : (c + 1) * 128],
                            rhs=xTall[:, e * NSH + ns : e * NSH + ns + 512],
                            start=(e == 0),
                            stop=(e == EC - 1),
                        )
                    kt = ktp.tile([128, 512], F32R, tag="kt")
                    nc.scalar.activation(
                        out=kt,
                        in_=kps,
                        func=mybir.ActivationFunctionType.Identity,
                        bias=bkTs[c],
                    )
                    ktiles.append(kt)

                # ---- V per node chunk: [128 n, 512 eout] (bf16) ----
                vtiles = []
                for c in range(EC):
                    vps = psw.tile([128, 512], F32, tag="work")
                    for e in range(EC):
                        nc.tensor.matmul(
                            vps,
                            lhsT=xTall[
                                :, e * NSH + ns + c * 128 : e * NSH + ns + (c + 1) * 128
                            ],
                            rhs=wvTs[e],
                            start=(e == 0),
                            stop=(e == EC - 1),
                        )
                    vt = vtp.tile([128, 512], BF16, tag="vt")
                    nc.vector.tensor_copy(vt, vps)
                    vtiles.append(vt)

                # ---- scores + softmax + AV, per head-quad q (4 heads) ----
                for q in range(2):
                    sps = psw.tile([128, 512], F32, tag="work")
                    for hp2 in range(2):
                        hp = q * 2 + hp2  # head pair == eout chunk of K
                        nc.tensor.matmul(
                            sps,
                            lhsT=qtbd[:, hp * 128 : (hp + 1) * 128],
                            rhs=ktiles[hp],
                            start=(hp2 == 0),
                            stop=(hp2 == 1),
                        )
                    expt = smp.tile([128, 512], F32, tag="exp")
                    sumt = smp.tile([128, 1], F32, tag="sum")
                    nc.scalar.activation(
                        out=expt,
                        in_=sps,
                        func=mybir.ActivationFunctionType.Exp,
                        accum_out=sumt,
                    )
                    rect = smp.tile([128, 1], F32, tag="rec")
                    nc.vector.reciprocal(out=rect, in_=sumt)
                    attnt = smp.tile([128, 512], BF16, tag="attn")
                    nc.vector.tensor_scalar_mul(attnt, expt, rect)

                    atps = psa.tile([128, 512], BF16, tag="atp")
                    for c in range(EC):
                        nc.tensor.transpose(
                            atps[:, c * 128 : (c + 1) * 128],
                            attnt[:, c * 128 : (c + 1) * 128],
                            identb,
                        )
                    attnT = smp.tile([128, 512], BF16, tag="attnT")
                    nc.vector.tensor_copy(attnT, atps)

                    # AV: accumulate OT blocks [64 d, 32 q] over node chunks
                    for hq in range(4):
                        h = 4 * q + hq
                        pcol = (h // 2) * 128 + g * 32
                        for c in range(EC):
                            nc.tensor.matmul(
                                otps[h % 2][:, pcol : pcol + 32],
                                lhsT=vtiles[c][:, h * HD : (h + 1) * HD],
                                rhs=attnT[:, c * 128 + hq * 32 : c * 128 + (hq + 1) * 32],
                                start=(c == 0),
                                stop=(c == EC - 1),
                            )

            emit_transpose_block(0)
            for g in range(GPC):
                if g + 1 < GPC:
                    emit_transpose_block(g + 1)
                emit_graph(g)

            # ---- output projection ----
            ot_sb = finp.tile([128, 512], F32R, tag="otsb")
            for p in range(2):
                nc.scalar.activation(
                    out=ot_sb[p * 64 : (p + 1) * 64, :],
                    in_=otps[p],
                    func=mybir.ActivationFunctionType.Copy,
                )
            xcps = psw.tile([128, 512], F32, tag="work")
            nc.tensor.matmul(xcps, lhsT=ones_r, rhs=bo_row, start=True, stop=False)
            for c in range(EC):
                nc.tensor.matmul(
                    xcps,
                    lhsT=ot_sb[:, c * 128 : (c + 1) * 128],
                    rhs=woTs[c],
                    start=False,
                    stop=(c == EC - 1),
                )
            xc_sb = finp.tile([128, 512], F32, tag="xc")
            nc.scalar.activation(
                out=xc_sb, in_=xcps, func=mybir.ActivationFunctionType.Copy
            )
            nc.sync.dma_start(out=out_d[:, :], in_=xc_sb)

    nc.finalize()
    return nc


def _host_prep(x, xcent_base, in_proj_w, in_proj_b, out_proj_w, out_proj_b):
    x = np.asarray(x, np.float32)
    ipw = np.asarray(in_proj_w, np.float32)
    ipb = np.asarray(in_proj_b, np.float32)
    wo = np.asarray(out_proj_w, np.float32)
    bo = np.asarray(out_proj_b, np.float32)
    xcb = np.asarray(xcent_base, np.float32)

    wq, wk, wv = ipw[0:E], ipw[E : 2 * E], ipw[2 * E : 3 * E]
    bq, bk, bv = ipb[0:E], ipb[E : 2 * E], ipb[2 * E : 3 * E]

    # query projection: identical for every graph; scale folded in
    qs = (xcb @ wq.T + bq) * np.float32(1.0 / np.sqrt(HD))  # [32, 512]
    qtbd = np.zeros((128, 512), np.float32)
    for hp in range(4):
        for a in range(2):
            h = 2 * hp + a
            m0 = hp * 128 + (hp % 2) * 64 + a * 32
            qtbd[a * 64 : (a + 1) * 64, m0 : m0 + 32] = qs[:, h * HD : (h + 1) * HD].T

    consts = dict(
        wkT=np.ascontiguousarray(wk.T),
        wvT=np.ascontiguousarray(wv.T),
        woT=np.ascontiguousarray(wo.T),
        qtbd=qtbd,
        bkT=np.ascontiguousarray(bk.reshape(EC, 128).T),
        ones=np.ones((1, 128), np.float32),
        ident=np.eye(128, dtype=np.float32),
        bo=np.ascontiguousarray((wo @ bv + bo)[None, :]),
    )
    return x, consts


def kernel(
    x,
    edge_index,
    batch,
    xcent_base,
    in_proj_w,
    in_proj_b,
    out_proj_w,
    out_proj_b,
):
    global LAST_RESULT
    x, consts = _host_prep(
        x, xcent_base, in_proj_w, in_proj_b, out_proj_w, out_proj_b
    )

    if "nc" not in _NC_CACHE:
        _NC_CACHE["nc"] = _build_nc()
    nc = _NC_CACHE["nc"]

    in_maps = [
        dict(consts, xs=np.ascontiguousarray(x[k * NSH : (k + 1) * NSH]))
        for k in range(NCORES)
    ]
    res = run_bass_kernel_spmd(nc, in_maps, list(range(NCORES)))
    LAST_RESULT = res

    xcent = np.concatenate(
        [res.results[k]["out"].reshape(GPC, RATIO, E) for k in range(NCORES)], axis=0
    ).astype(np.float32)

    batch = np.asarray(batch)
    batchcent = np.repeat(np.arange(B, dtype=batch.dtype), RATIO)
    return xcent, batchcent


# revision 8
# speedup vs baseline: 1.1201x; 1.0613x over previous
"""BipartPool (masked bipartite cluster attention) on 8 Trainium2 NeuronCores.

Problem (from reference):
    B=32 graphs x 512 nodes, E=512, H=8 heads (head_dim 64), RATIO=32 cluster
    queries per graph (identical seeds xcent_base for every graph).
    xcent = MHA(q=tile(xcent_base,B), k=v=x, mask=block-diagonal by graph)

Strategy: data-parallel over graphs, 4 graphs per core. The block-diagonal
mask means each (graph, query-block) only attends to its own 512 nodes, so
scores are computed per graph: [32 q, 512 n] per head instead of the
reference's dense [1024, 16384].

Per-core device pipeline (bf16 operands into the PE, fp32 PSUM accumulation):
  1. x shard loaded via SWDGE cast-DMA (fp32 HBM -> bf16 SBUF), PE-transposed
     (single-pass bf16) into xT [512 e, 2048 n].
  2. KT = wkT-chunks x xT   -> [512 eout, 512 n] per graph (+bk via ACT).
     V  = xT-chunks x wvT   -> [512 n, 512 eout] per graph.
  3. scores via zero-padded block-diagonal Q: two accumulating matmuls per
     (graph, head-quad) -> psum [128 = (4h x 32q), 512 n].
  4. softmax over the free axis without max-subtraction (scores are O(1)):
     ACT Exp with accum_out row-sums, DVE reciprocal + per-partition scale.
  5. attn PE-transposed (bf16); AV as per-(graph, head-pair) quadrant
     matmuls: lhsT = V [128 n, 128 (2h d)], rhs = attnT [128 n, 64 (2h q)],
     diagonal quadrants accumulated over node chunks, copied into OT layout.
  6. out-projection (fp32r): OT-chunks x woT + bias outer-product.

Host precomputes (weight-only work): wk/wv transposes cast to bf16, woT in
fp32, the query projection Q = xcent_base @ wq.T + bq (identical for every
graph) scaled by 1/sqrt(64) in block-diagonal layout, and the folded output
bias bo' = Wo @ bv + bo (valid because attention rows sum to 1).
"""

import numpy as np
import ml_dtypes

import concourse.bacc as bacc
import concourse.mybir as mybir
from concourse.tile import TileContext
from concourse.bass_utils import run_bass_kernel_spmd
from concourse.masks import make_identity

F32 = mybir.dt.float32
F32R = mybir.dt.float32r
BF16 = mybir.dt.bfloat16

B, RATIO, E, H = 32, 32, 512, 8
HD = E // H  # 64
NODES_PER_GRAPH = 512
N = B * NODES_PER_GRAPH  # 16384
NCORES = 8
GPC = B // NCORES  # graphs per core = 4
NSH = GPC * NODES_PER_GRAPH  # nodes per core = 2048
EC = E // 128  # feature chunks = 4

_NC_CACHE = {}
LAST_RESULT = None  # test harness reads exec_time_ns from here


def _build_nc():
    nc = bacc.Bacc(None, target_bir_lowering=False)

    xs_d = nc.dram_tensor("xs", [NSH, E], F32, kind="ExternalInput")
    wkT_d = nc.dram_tensor("wkT", [E, E], BF16, kind="ExternalInput")
    wvT_d = nc.dram_tensor("wvT", [E, E], BF16, kind="ExternalInput")
    woT_d = nc.dram_tensor("woT", [E, E], F32R, kind="ExternalInput")
    qtbd_d = nc.dram_tensor("qtbd", [128, 512], BF16, kind="ExternalInput")
    bkT_d = nc.dram_tensor("bkT", [128, EC], F32, kind="ExternalInput")
    ones_d = nc.dram_tensor("ones", [1, 128], F32R, kind="ExternalInput")
    bo_d = nc.dram_tensor("bo", [1, E], F32R, kind="ExternalInput")
    out_d = nc.dram_tensor("out", [GPC * RATIO, E], F32, kind="ExternalOutput")

    with TileContext(nc) as tc:
        with (
            tc.tile_pool(name="const", bufs=1) as const,
            tc.tile_pool(name="xta", bufs=1) as xta,
            tc.tile_pool(name="xn", bufs=3) as xnp,
            tc.tile_pool(name="kt", bufs=6) as ktp,
            tc.tile_pool(name="vt", bufs=8) as vtp,
            tc.tile_pool(name="sm", bufs=4) as smp,
            tc.tile_pool(name="fin", bufs=1) as finp,
            tc.tile_pool(name="psw", bufs=2, space="PSUM") as psw,
            tc.tile_pool(name="pst", bufs=2, space="PSUM") as pst,
            tc.tile_pool(name="psa", bufs=2, space="PSUM") as psa,
            tc.tile_pool(name="psq", bufs=2, space="PSUM") as psq,
        ):
            # ---- constants / weights ----
            identb = const.tile([128, 128], BF16, tag="idb")
            make_identity(nc, identb)

            qtbd = const.tile([128, 512], BF16, tag="qtbd")
            nc.sync.dma_start(out=qtbd, in_=qtbd_d[:, :])
            bkT4 = const.tile([128, EC], F32, tag="bk")
            nc.sync.dma_start(out=bkT4, in_=bkT_d[:, :])
            bkTs = [bkT4[:, c : c + 1] for c in range(EC)]
            ones_r = const.tile([1, 128], F32R, tag="ones")
            nc.sync.dma_start(out=ones_r, in_=ones_d[:, :])
            bo_row = const.tile([1, E], F32R, tag="bo")
            nc.sync.dma_start(out=bo_row, in_=bo_d[:, :])

            # each weight matrix in ONE DMA: [512, 512] -> [128, (c, 512)]
            wkTs, wvTs, woTs = [], [], []
            for name, dram, lst, dt in (
                ("wk", wkT_d, wkTs, BF16),
                ("wv", wvT_d, wvTs, BF16),
                ("wo", woT_d, woTs, F32R),
            ):
                t4 = const.tile([128, EC * E], dt, tag=f"{name}all", name=f"{name}all")
                nc.scalar.dma_start(
                    out=t4.rearrange("p (c e) -> p c e", c=EC),
                    in_=dram.ap().rearrange("(c p) e -> p c e", p=128),
                )
                for c in range(EC):
                    lst.append(t4[:, c * E : (c + 1) * E])

            # xT: [512 e, 2048 n] packed as [128, EC * NSH], bf16
            xTall = xta.tile([128, EC * NSH], BF16, tag="xT")
            xT3 = xTall.rearrange("p (c n) -> p c n", c=EC)

            # OT in SBUF: [128 e-in-chunk, (chunk|graph|q) 512], fp32r for
            # the output projection; filled by quadrant copies from psum
            ot_sb = finp.tile([128, 512], F32R, tag="otsb")

            def emit_transpose_block(g):
                # SWDGE cast-DMA: fp32 HBM -> bf16 SBUF. Graph 0 split in 4
                # chunks so the PE can start on the first 128 rows sooner.
                nparts = 4 if g == 0 else 1
                xns = []
                for part in range(nparts):
                    rows = 512 // nparts
                    xn = xnp.tile(
                        [128, (rows // 128) * E], BF16, tag="xn", name=f"xn{g}_{part}"
                    )
                    src = xs_d.ap()[
                        g * 512 + part * rows : g * 512 + (part + 1) * rows, :
                    ].rearrange("(r p) e -> p r e", p=128)
                    nc.gpsimd.dma_start(
                        out=xn.rearrange("p (r e) -> p r e", e=E), in_=src
                    )
                    xns.append(xn)
                for r in range(4):
                    nb = 4 * g + r
                    xn = xns[r] if nparts == 4 else xns[0]
                    roff = 0 if nparts == 4 else r * E
                    pt = pst.tile([128, 512], BF16, tag="xtp", name="pt")
                    for c in range(EC):
                        nc.tensor.transpose(
                            pt[:, c * 128 : (c + 1) * 128],
                            xn[:, roff + c * 128 : roff + (c + 1) * 128],
                            identb,
                        )
                    # one strided copy: psum block c -> xT[c][:, nb*128:...]
                    # alternate ACT / DVE to balance engine load
                    if nb % 2 == 0:
                        nc.scalar.activation(
                            out=xT3[:, :, nb * 128 : (nb + 1) * 128],
                            in_=pt.rearrange("p (c n) -> p c n", c=EC),
                            func=mybir.ActivationFunctionType.Copy,
                        )
                    else:
                        nc.vector.tensor_copy(
                            xT3[:, :, nb * 128 : (nb + 1) * 128],
                            pt.rearrange("p (c n) -> p c n", c=EC),
                        )

            def emit_graph(g):
                ns = g * NODES_PER_GRAPH  # node offset in xT free dim

                # ---- K^T per eout chunk: [128 eout, 512 n] ----
                ktiles = []
                for c in range(EC):
                    kps = psw.tile([128, 512], F32, tag="work", name="kps")
                    for e in range(EC):
                        nc.tensor.matmul(
                            kps,
                            lhsT=wkTs[e][:, c * 128 : (c + 1) * 128],
                            rhs=xTall[:, e * NSH + ns : e * NSH + ns + 512],
                            start=(e == 0),
                            stop=(e == EC - 1),
                        )
                    kt = ktp.tile([128, 512], BF16, tag="kt", name="kt")
                    nc.scalar.activation(
                        out=kt,
                        in_=kps,
                        func=mybir.ActivationFunctionType.Identity,
                        bias=bkTs[c],
                    )
                    ktiles.append(kt)

                # ---- V per node chunk: [128 n, 512 eout] (bf16) ----
                vtiles = []
                for c in range(EC):
                    vps = psw.tile([128, 512], F32, tag="work", name="vps")
                    for e in range(EC):
                        nc.tensor.matmul(
                            vps,
                            lhsT=xTall[
                                :, e * NSH + ns + c * 128 : e * NSH + ns + (c + 1) * 128
                            ],
                            rhs=wvTs[e],
                            start=(e == 0),
                            stop=(e == EC - 1),
                        )
                    vt = vtp.tile([128, 512], BF16, tag="vt", name="vt")
                    nc.vector.tensor_copy(vt, vps)
                    vtiles.append(vt)

                # ---- scores + softmax, per head-quad q (4 heads) ----
                attnTs = []
                for q in range(2):
                    sps = psw.tile([128, 512], F32, tag="work", name="sps")
                    for hp2 in range(2):
                        hp = q * 2 + hp2  # head pair == eout chunk of K
                        nc.tensor.matmul(
                            sps,
                            lhsT=qtbd[:, hp * 128 : (hp + 1) * 128],
                            rhs=ktiles[hp],
                            start=(hp2 == 0),
                            stop=(hp2 == 1),
                        )
                    expt = smp.tile([128, 512], F32, tag="exp", name="expt")
                    sumt = smp.tile([128, 1], F32, tag="sum", name="sumt")
                    nc.scalar.activation(
                        out=expt,
                        in_=sps,
                        func=mybir.ActivationFunctionType.Exp,
                        accum_out=sumt,
                    )
                    rect = smp.tile([128, 1], F32, tag="rec", name="rect")
                    nc.vector.reciprocal(out=rect, in_=sumt)
                    attnt = smp.tile([128, 512], BF16, tag="attn", name="attnt")
                    nc.vector.tensor_scalar_mul(attnt, expt, rect)

                    atps = psa.tile([128, 512], BF16, tag="atp", name="atps")
                    for c in range(EC):
                        nc.tensor.transpose(
                            atps[:, c * 128 : (c + 1) * 128],
                            attnt[:, c * 128 : (c + 1) * 128],
                            identb,
                        )
                    attnT = smp.tile([128, 512], BF16, tag="attnT", name="attnT")
                    nc.vector.tensor_copy(attnT, atps)
                    attnTs.append(attnT)

                # ---- AV quadrants: per head-pair ce ----
                # lhsT = V [128 n, 128 (2h d)], rhs = attnT two-head cols
                # [128 n, 64]; diagonal quadrants accumulate over chunks c
                for ce in range(EC):
                    avq = psq.tile([128, 64], F32, tag="avq", name="avq")
                    aT = attnTs[ce // 2]
                    hq0 = 2 * (ce % 2)  # first head's col block within quad
                    for c in range(EC):
                        nc.tensor.matmul(
                            avq,
                            lhsT=vtiles[c][:, ce * 128 : (ce + 1) * 128],
                            rhs=aT[:, c * 128 + hq0 * 32 : c * 128 + (hq0 + 2) * 32],
                            start=(c == 0),
                            stop=(c == EC - 1),
                        )
                    # extract diagonal quadrants into OT layout
                    for a in range(2):
                        nc.scalar.activation(
                            out=ot_sb[
                                a * 64 : (a + 1) * 64,
                                ce * 128 + g * 32 : ce * 128 + (g + 1) * 32,
                            ],
                            in_=avq[a * 64 : (a + 1) * 64, a * 32 : (a + 1) * 32],
                            func=mybir.ActivationFunctionType.Copy,
                        )

            emit_transpose_block(0)
            for g in range(GPC):
                if g + 1 < GPC:
                    emit_transpose_block(g + 1)
                emit_graph(g)

            # ---- output projection (fp32r) ----
            xcps = psw.tile([128, 512], F32, tag="work", name="xcps")
            nc.tensor.matmul(xcps, lhsT=ones_r, rhs=bo_row, start=True, stop=False)
            for c in range(EC):
                nc.tensor.matmul(
                    xcps,
                    lhsT=ot_sb[:, c * 128 : (c + 1) * 128],
                    rhs=woTs[c],
                    start=False,
                    stop=(c == EC - 1),
                )
            xc_sb = finp.tile([128, 512], F32, tag="xc")
            nc.scalar.activation(
                out=xc_sb, in_=xcps, func=mybir.ActivationFunctionType.Copy
            )
            nc.sync.dma_start(out=out_d[:, :], in_=xc_sb)

    nc.finalize()
    return nc


def _host_prep(x, xcent_base, in_proj_w, in_proj_b, out_proj_w, out_proj_b):
    x = np.asarray(x, np.float32)
    ipw = np.asarray(in_proj_w, np.float32)
    ipb = np.asarray(in_proj_b, np.float32)
    wo = np.asarray(out_proj_w, np.float32)
    bo = np.asarray(out_proj_b, np.float32)
    xcb = np.asarray(xcent_base, np.float32)

    wq, wk, wv = ipw[0:E], ipw[E : 2 * E], ipw[2 * E : 3 * E]
    bq, bk, bv = ipb[0:E], ipb[E : 2 * E], ipb[2 * E : 3 * E]

    # query projection: identical for every graph; scale folded in
    qs = (xcb @ wq.T + bq) * np.float32(1.0 / np.sqrt(HD))  # [32, 512]
    qtbd = np.zeros((128, 512), np.float32)
    for hp in range(4):
        for a in range(2):
            h = 2 * hp + a
            m0 = hp * 128 + (hp % 2) * 64 + a * 32
            qtbd[a * 64 : (a + 1) * 64, m0 : m0 + 32] = qs[:, h * HD : (h + 1) * HD].T

    bf = ml_dtypes.bfloat16
    consts = dict(
        wkT=np.ascontiguousarray(wk.T).astype(bf),
        wvT=np.ascontiguousarray(wv.T).astype(bf),
        woT=np.ascontiguousarray(wo.T),
        qtbd=qtbd.astype(bf),
        bkT=np.ascontiguousarray(bk.reshape(EC, 128).T),
        ones=np.ones((1, 128), np.float32),
        bo=np.ascontiguousarray((wo @ bv + bo)[None, :]),
    )
    return x, consts


def kernel(
    x,
    edge_index,
    batch,
    xcent_base,
    in_proj_w,
    in_proj_b,
    out_proj_w,
    out_proj_b,
):
    global LAST_RESULT
    x, consts = _host_prep(
        x, xcent_base, in_proj_w, in_proj_b, out_proj_w, out_proj_b
    )

    if "nc" not in _NC_CACHE:
        _NC_CACHE["nc"] = _build_nc()
    nc = _NC_CACHE["nc"]

    in_maps = [
        dict(consts, xs=np.ascontiguousarray(x[k * NSH : (k + 1) * NSH]))
        for k in range(NCORES)
    ]
    res = run_bass_kernel_spmd(nc, in_maps, list(range(NCORES)))
    LAST_RESULT = res

    xcent = np.concatenate(
        [res.results[k]["out"].reshape(GPC, RATIO, E) for k in range(NCORES)], axis=0
    ).astype(np.float32)

    batch = np.asarray(batch)
    batchcent = np.repeat(np.arange(B, dtype=batch.dtype), RATIO)
    return xcent, batchcent


# revision 9
# speedup vs baseline: 1.3714x; 1.2244x over previous
"""BipartPool (masked bipartite cluster attention) on 8 Trainium2 NeuronCores.

Problem (from reference):
    B=32 graphs x 512 nodes, E=512, H=8 heads (head_dim 64), RATIO=32 cluster
    queries per graph (identical seeds xcent_base for every graph).
    xcent = MHA(q=tile(xcent_base,B), k=v=x, mask=block-diagonal by graph)

Strategy: data-parallel over graphs, 4 graphs per core. The block-diagonal
mask means each (graph, query-block) only attends to its own 512 nodes, so
scores are computed per graph: [32 q, 512 n] per head instead of the
reference's dense [1024, 16384].

Per-core device pipeline (bf16 operands into the PE, fp32 PSUM accumulation):
  1. x shard loaded via SWDGE cast-DMA (fp32 HBM -> bf16 SBUF), PE-transposed
     (single-pass bf16) into xT [512 e, 2048 n].
  2. KT = wkT-chunks x xT   -> [512 eout, 512 n] per graph (+bk via ACT).
     V  = xT-chunks x wvT   -> [512 n, 512 eout] per graph.
  3. scores via zero-padded block-diagonal Q: two accumulating matmuls per
     (graph, head-quad) -> psum [128 = (4h x 32q), 512 n].
  4. softmax over the free axis without max-subtraction (scores are O(1)):
     ACT Exp with accum_out row-sums, DVE reciprocal + per-partition scale.
  5. attn PE-transposed (bf16); AV as per-(graph, head-pair) quadrant
     matmuls: lhsT = V [128 n, 128 (2h d)], rhs = attnT [128 n, 64 (2h q)],
     diagonal quadrants accumulated over node chunks, copied into OT layout.
  6. out-projection (fp32r): OT-chunks x woT + bias outer-product.

Host precomputes (weight-only work): wk/wv transposes cast to bf16, woT in
fp32, the query projection Q = xcent_base @ wq.T + bq (identical for every
graph) scaled by 1/sqrt(64) in block-diagonal layout, and the folded output
bias bo' = Wo @ bv + bo (valid because attention rows sum to 1).
"""

import numpy as np
import ml_dtypes

import concourse.bacc as bacc
import concourse.mybir as mybir
from concourse.tile import TileContext
from concourse.bass_utils import run_bass_kernel_spmd
from concourse.masks import make_identity

F32 = mybir.dt.float32
F32R = mybir.dt.float32r
BF16 = mybir.dt.bfloat16

B, RATIO, E, H = 32, 32, 512, 8
HD = E // H  # 64
NODES_PER_GRAPH = 512
N = B * NODES_PER_GRAPH  # 16384
NCORES = 8
GPC = B // NCORES  # graphs per core = 4
NSH = GPC * NODES_PER_GRAPH  # nodes per core = 2048
EC = E // 128  # feature chunks = 4

_NC_CACHE = {}
LAST_RESULT = None  # test harness reads exec_time_ns from here


def _build_nc():
    nc = bacc.Bacc(None, target_bir_lowering=False)

    xs_d = nc.dram_tensor("xs", [NSH, E], F32, kind="ExternalInput")
    wvT_d = nc.dram_tensor("wvT", [E, E], BF16, kind="ExternalInput")
    woT_d = nc.dram_tensor("woT", [E, E], F32R, kind="ExternalInput")
    qkT_d = nc.dram_tensor("qkT", [E, 256], BF16, kind="ExternalInput")
    ones_d = nc.dram_tensor("ones", [1, 128], F32R, kind="ExternalInput")
    bo_d = nc.dram_tensor("bo", [1, E], F32R, kind="ExternalInput")
    out_d = nc.dram_tensor("out", [GPC * RATIO, E], F32, kind="ExternalOutput")

    with TileContext(nc) as tc:
        with (
            tc.tile_pool(name="const", bufs=1) as const,
            tc.tile_pool(name="xta", bufs=1) as xta,
            tc.tile_pool(name="xn", bufs=3) as xnp,
            tc.tile_pool(name="vt", bufs=8) as vtp,
            tc.tile_pool(name="sm", bufs=4) as smp,
            tc.tile_pool(name="fin", bufs=1) as finp,
            tc.tile_pool(name="psw", bufs=3, space="PSUM") as psw,
            tc.tile_pool(name="pst", bufs=2, space="PSUM") as pst,
            tc.tile_pool(name="psa", bufs=2, space="PSUM") as psa,
            tc.tile_pool(name="psq", bufs=1, space="PSUM") as psq,
        ):
            # ---- constants / weights ----
            identb = const.tile([128, 128], BF16, tag="idb")
            make_identity(nc, identb)

            # QK = Qs @ wk folded on host: [512 e, 256 (quad, h, q)]
            qkt = const.tile([128, EC * 256], BF16, tag="qkt")
            nc.sync.dma_start(
                out=qkt.rearrange("p (c j) -> p c j", c=EC),
                in_=qkT_d.ap().rearrange("(c p) j -> p c j", p=128),
            )
            ones_r = const.tile([1, 128], F32R, tag="ones")
            nc.sync.dma_start(out=ones_r, in_=ones_d[:, :])
            bo_row = const.tile([1, E], F32R, tag="bo")
            nc.sync.dma_start(out=bo_row, in_=bo_d[:, :])

            # each weight matrix in ONE DMA: [512, 512] -> [128, (c, 512)]
            wvTs, woTs = [], []
            for name, dram, lst, dt in (
                ("wv", wvT_d, wvTs, BF16),
                ("wo", woT_d, woTs, F32R),
            ):
                t4 = const.tile([128, EC * E], dt, tag=f"{name}all", name=f"{name}all")
                nc.scalar.dma_start(
                    out=t4.rearrange("p (c e) -> p c e", c=EC),
                    in_=dram.ap().rearrange("(c p) e -> p c e", p=128),
                )
                for c in range(EC):
                    lst.append(t4[:, c * E : (c + 1) * E])

            # xT: [512 e, 2048 n] packed as [128, EC * NSH], bf16
            xTall = xta.tile([128, EC * NSH], BF16, tag="xT")
            xT3 = xTall.rearrange("p (c n) -> p c n", c=EC)

            # OT in SBUF: [128 e-in-chunk, (chunk|graph|q) 512], fp32r for
            # the output projection; filled by quadrant copies from psum
            ot_sb = finp.tile([128, 512], F32R, tag="otsb")

            def emit_transpose_block(g):
                # plain fp32 HWDGE loads (fast); cast to bf16 on ACT/DVE.
                # Graph 0 split in 4 chunks so the PE can start sooner.
                nparts = 4 if g == 0 else 1
                xns = []
                for part in range(nparts):
                    rows = 512 // nparts
                    xn = xnp.tile(
                        [128, (rows // 128) * E], F32, tag="xn", name=f"xn{g}_{part}"
                    )
                    src = xs_d.ap()[
                        g * 512 + part * rows : g * 512 + (part + 1) * rows, :
                    ].rearrange("(r p) e -> p r e", p=128)
                    nc.sync.dma_start(
                        out=xn.rearrange("p (r e) -> p r e", e=E), in_=src
                    )
                    xns.append(xn)
                for r in range(4):
                    nb = 4 * g + r
                    xn = xns[r] if nparts == 4 else xns[0]
                    roff = 0 if nparts == 4 else r * E
                    xnb = xnp.tile([128, E], BF16, tag="xnb", name="xnb")
                    if nb % 2 == 0:
                        nc.scalar.activation(
                            out=xnb,
                            in_=xn[:, roff : roff + E],
                            func=mybir.ActivationFunctionType.Copy,
                        )
                    else:
                        nc.vector.tensor_copy(xnb, xn[:, roff : roff + E])
                    roff = 0
                    xn = xnb
                    pt = pst.tile([128, 512], BF16, tag="xtp", name="pt")
                    for c in range(EC):
                        nc.tensor.transpose(
                            pt[:, c * 128 : (c + 1) * 128],
                            xn[:, roff + c * 128 : roff + (c + 1) * 128],
                            identb,
                        )
                    # one strided copy: psum block c -> xT[c][:, nb*128:...]
                    # alternate ACT / DVE to balance engine load
                    if nb % 2 == 0:
                        nc.scalar.activation(
                            out=xT3[:, :, nb * 128 : (nb + 1) * 128],
                            in_=pt.rearrange("p (c n) -> p c n", c=EC),
                            func=mybir.ActivationFunctionType.Copy,
                        )
                    else:
                        nc.vector.tensor_copy(
                            xT3[:, :, nb * 128 : (nb + 1) * 128],
                            pt.rearrange("p (c n) -> p c n", c=EC),
                        )

            def emit_graph(g):
                ns = g * NODES_PER_GRAPH  # node offset in xT free dim

                # ---- V per node chunk: [128 n, 512 eout] (bf16) ----
                vtiles = []
                for c in range(EC):
                    vps = psw.tile([128, 512], F32, tag="work", name="vps")
                    for e in range(EC):
                        nc.tensor.matmul(
                            vps,
                            lhsT=xTall[
                                :, e * NSH + ns + c * 128 : e * NSH + ns + (c + 1) * 128
                            ],
                            rhs=wvTs[e],
                            start=(e == 0),
                            stop=(e == EC - 1),
                        )
                    vt = vtp.tile([128, 512], BF16, tag="vt", name="vt")
                    nc.vector.tensor_copy(vt, vps)
                    vtiles.append(vt)

                # ---- scores + softmax, per head-quad q (4 heads) ----
                # scores = (Qs @ wk) @ x.T directly from xT; the key bias
                # bk only shifts each row by a constant, which softmax
                # ignores, so it is dropped
                attnTs = []
                for q in range(2):
                    sps = psw.tile([128, 512], F32, tag="work", name="sps")
                    for c in range(EC):
                        nc.tensor.matmul(
                            sps,
                            lhsT=qkt[:, c * 256 + q * 128 : c * 256 + (q + 1) * 128],
                            rhs=xTall[:, c * NSH + ns : c * NSH + ns + 512],
                            start=(c == 0),
                            stop=(c == EC - 1),
                        )
                    expt = smp.tile([128, 512], F32, tag="exp", name="expt")
                    sumt = smp.tile([128, 1], F32, tag="sum", name="sumt")
                    nc.scalar.activation(
                        out=expt,
                        in_=sps,
                        func=mybir.ActivationFunctionType.Exp,
                        accum_out=sumt,
                    )
                    rect = smp.tile([128, 1], F32, tag="rec", name="rect")
                    nc.vector.reciprocal(out=rect, in_=sumt)
                    attnt = smp.tile([128, 512], BF16, tag="attn", name="attnt")
                    nc.vector.tensor_scalar_mul(attnt, expt, rect)

                    atps = psa.tile([128, 512], BF16, tag="atp", name="atps")
                    for c in range(EC):
                        nc.tensor.transpose(
                            atps[:, c * 128 : (c + 1) * 128],
                            attnt[:, c * 128 : (c + 1) * 128],
                            identb,
                        )
                    attnT = smp.tile([128, 512], BF16, tag="attnT", name="attnT")
                    nc.vector.tensor_copy(attnT, atps)
                    attnTs.append(attnT)

                # ---- AV quadrants: per head-pair ce ----
                # lhsT = V [128 n, 128 (2h d)], rhs = attnT two-head cols
                # [128 n, 64]; diagonal quadrants accumulate over chunks c
                for ce in range(EC):
                    avq = psq.tile([128, 64], F32, tag="avq", name="avq")
                    aT = attnTs[ce // 2]
                    hq0 = 2 * (ce % 2)  # first head's col block within quad
                    for c in range(EC):
                        nc.tensor.matmul(
                            avq,
                            lhsT=vtiles[c][:, ce * 128 : (ce + 1) * 128],
                            rhs=aT[:, c * 128 + hq0 * 32 : c * 128 + (hq0 + 2) * 32],
                            start=(c == 0),
                            stop=(c == EC - 1),
                        )
                    # extract diagonal quadrants into OT layout
                    for a in range(2):
                        nc.scalar.activation(
                            out=ot_sb[
                                a * 64 : (a + 1) * 64,
                                ce * 128 + g * 32 : ce * 128 + (g + 1) * 32,
                            ],
                            in_=avq[a * 64 : (a + 1) * 64, a * 32 : (a + 1) * 32],
                            func=mybir.ActivationFunctionType.Copy,
                        )

            emit_transpose_block(0)
            for g in range(GPC):
                if g + 1 < GPC:
                    emit_transpose_block(g + 1)
                emit_graph(g)

            # ---- output projection (fp32r) ----
            xcps = psw.tile([128, 512], F32, tag="work", name="xcps")
            nc.tensor.matmul(xcps, lhsT=ones_r, rhs=bo_row, start=True, stop=False)
            for c in range(EC):
                nc.tensor.matmul(
                    xcps,
                    lhsT=ot_sb[:, c * 128 : (c + 1) * 128],
                    rhs=woTs[c],
                    start=False,
                    stop=(c == EC - 1),
                )
            xc_sb = finp.tile([128, 512], F32, tag="xc")
            nc.scalar.activation(
                out=xc_sb, in_=xcps, func=mybir.ActivationFunctionType.Copy
            )
            nc.sync.dma_start(out=out_d[:, :], in_=xc_sb)

    nc.finalize()
    return nc


def _host_prep(x, xcent_base, in_proj_w, in_proj_b, out_proj_w, out_proj_b):
    x = np.asarray(x, np.float32)
    ipw = np.asarray(in_proj_w, np.float32)
    ipb = np.asarray(in_proj_b, np.float32)
    wo = np.asarray(out_proj_w, np.float32)
    bo = np.asarray(out_proj_b, np.float32)
    xcb = np.asarray(xcent_base, np.float32)

    wq, wk, wv = ipw[0:E], ipw[E : 2 * E], ipw[2 * E : 3 * E]
    bq, bk, bv = ipb[0:E], ipb[E : 2 * E], ipb[2 * E : 3 * E]

    # query projection (identical for every graph), folded with wk:
    # QK_h = Qs_h @ wk_h  ->  scores_h = QK_h @ x.T
    qs = (xcb @ wq.T + bq) * np.float32(1.0 / np.sqrt(HD))  # [32, 512]
    qkT = np.zeros((E, 256), np.float32)
    for h in range(H):
        qk_h = qs[:, h * HD : (h + 1) * HD] @ wk[h * HD : (h + 1) * HD, :]  # [32, E]
        q_, hq = divmod(h, 4)
        qkT[:, q_ * 128 + hq * 32 : q_ * 128 + (hq + 1) * 32] = qk_h.T

    bf = ml_dtypes.bfloat16
    consts = dict(
        wvT=np.ascontiguousarray(wv.T).astype(bf),
        woT=np.ascontiguousarray(wo.T),
        qkT=qkT.astype(bf),
        ones=np.ones((1, 128), np.float32),
        bo=np.ascontiguousarray((wo @ bv + bo)[None, :]),
    )
    return x, consts


def kernel(
    x,
    edge_index,
    batch,
    xcent_base,
    in_proj_w,
    in_proj_b,
    out_proj_w,
    out_proj_b,
):
    global LAST_RESULT
    x, consts = _host_prep(
        x, xcent_base, in_proj_w, in_proj_b, out_proj_w, out_proj_b
    )

    if "nc" not in _NC_CACHE:
        _NC_CACHE["nc"] = _build_nc()
    nc = _NC_CACHE["nc"]

    in_maps = [
        dict(consts, xs=np.ascontiguousarray(x[k * NSH : (k + 1) * NSH]))
        for k in range(NCORES)
    ]
    res = run_bass_kernel_spmd(nc, in_maps, list(range(NCORES)))
    LAST_RESULT = res

    xcent = np.concatenate(
        [res.results[k]["out"].reshape(GPC, RATIO, E) for k in range(NCORES)], axis=0
    ).astype(np.float32)

    batch = np.asarray(batch)
    batchcent = np.repeat(np.arange(B, dtype=batch.dtype), RATIO)
    return xcent, batchcent


# revision 11
# speedup vs baseline: 1.3805x; 1.0067x over previous
"""BipartPool (masked bipartite cluster attention) on 8 Trainium2 NeuronCores.

Problem (from reference):
    B=32 graphs x 512 nodes, E=512, H=8 heads (head_dim 64), RATIO=32 cluster
    queries per graph (identical seeds xcent_base for every graph).
    xcent = MHA(q=tile(xcent_base,B), k=v=x, mask=block-diagonal by graph)

Strategy: data-parallel over graphs, 4 graphs per core. The block-diagonal
mask means each (graph, query-block) only attends to its own 512 nodes, so
scores are computed per graph: [32 q, 512 n] per head instead of the
reference's dense [1024, 16384].

Per-core device pipeline (bf16 operands into the PE, fp32 PSUM accumulation):
  1. x shard loaded via SWDGE cast-DMA (fp32 HBM -> bf16 SBUF), PE-transposed
     (single-pass bf16) into xT [512 e, 2048 n].
  2. KT = wkT-chunks x xT   -> [512 eout, 512 n] per graph (+bk via ACT).
     V  = xT-chunks x wvT   -> [512 n, 512 eout] per graph.
  3. scores via zero-padded block-diagonal Q: two accumulating matmuls per
     (graph, head-quad) -> psum [128 = (4h x 32q), 512 n].
  4. softmax over the free axis without max-subtraction (scores are O(1)):
     ACT Exp with accum_out row-sums, DVE reciprocal + per-partition scale.
  5. attn PE-transposed (bf16); AV as per-(graph, head-pair) quadrant
     matmuls: lhsT = V [128 n, 128 (2h d)], rhs = attnT [128 n, 64 (2h q)],
     diagonal quadrants accumulated over node chunks, copied into OT layout.
  6. out-projection (fp32r): OT-chunks x woT + bias outer-product.

Host precomputes (weight-only work): wk/wv transposes cast to bf16, woT in
fp32, the query projection Q = xcent_base @ wq.T + bq (identical for every
graph) scaled by 1/sqrt(64) in block-diagonal layout, and the folded output
bias bo' = Wo @ bv + bo (valid because attention rows sum to 1).
"""

import numpy as np
import ml_dtypes

import concourse.bacc as bacc
import concourse.mybir as mybir
from concourse.tile import TileContext
from concourse.bass_utils import run_bass_kernel_spmd
from concourse.masks import make_identity

F32 = mybir.dt.float32
F32R = mybir.dt.float32r
BF16 = mybir.dt.bfloat16

B, RATIO, E, H = 32, 32, 512, 8
HD = E // H  # 64
NODES_PER_GRAPH = 512
N = B * NODES_PER_GRAPH  # 16384
NCORES = 8
GPC = B // NCORES  # graphs per core = 4
NSH = GPC * NODES_PER_GRAPH  # nodes per core = 2048
EC = E // 128  # feature chunks = 4

_NC_CACHE = {}
LAST_RESULT = None  # test harness reads exec_time_ns from here


def _build_nc():
    nc = bacc.Bacc(None, target_bir_lowering=False)

    xs_d = nc.dram_tensor("xs", [NSH, E], F32, kind="ExternalInput")
    wvT_d = nc.dram_tensor("wvT", [E, E], BF16, kind="ExternalInput")
    woT_d = nc.dram_tensor("woT", [E, E], BF16, kind="ExternalInput")
    qkT_d = nc.dram_tensor("qkT", [E, 256], BF16, kind="ExternalInput")
    ones_d = nc.dram_tensor("ones", [1, 128], BF16, kind="ExternalInput")
    bo_d = nc.dram_tensor("bo", [1, E], BF16, kind="ExternalInput")
    out_d = nc.dram_tensor("out", [GPC * RATIO, E], F32, kind="ExternalOutput")

    with TileContext(nc) as tc:
        with (
            tc.tile_pool(name="const", bufs=1) as const,
            tc.tile_pool(name="xta", bufs=1) as xta,
            tc.tile_pool(name="xn", bufs=3) as xnp,
            tc.tile_pool(name="vt", bufs=8) as vtp,
            tc.tile_pool(name="sm", bufs=4) as smp,
            tc.tile_pool(name="fin", bufs=2) as finp,
            tc.tile_pool(name="psw", bufs=2, space="PSUM") as psw,
            tc.tile_pool(name="pst", bufs=3, space="PSUM") as pst,
            tc.tile_pool(name="psq", bufs=1, space="PSUM") as psq,
            tc.tile_pool(name="psx", bufs=2, space="PSUM") as psx,
        ):
            # ---- constants / weights ----
            identb = const.tile([128, 128], BF16, tag="idb")
            make_identity(nc, identb)

            # weight/const tiles; DMAs are issued later, ordered so the
            # first graph's x data wins the early HBM bandwidth
            qkt = const.tile([128, EC * 256], BF16, tag="qkt")
            ones_r = const.tile([1, 128], BF16, tag="ones")
            bo_row = const.tile([1, E], BF16, tag="bo")
            wvT4 = const.tile([128, EC * E], BF16, tag="wvall")
            woT4 = const.tile([128, EC * E], BF16, tag="woall")
            wvTs = [wvT4[:, c * E : (c + 1) * E] for c in range(EC)]
            woTs = [woT4[:, c * E : (c + 1) * E] for c in range(EC)]

            def emit_const_dmas():
                nc.scalar.dma_start(
                    out=wvT4.rearrange("p (c e) -> p c e", c=EC),
                    in_=wvT_d.ap().rearrange("(c p) e -> p c e", p=128),
                )
                nc.sync.dma_start(
                    out=qkt.rearrange("p (c j) -> p c j", c=EC),
                    in_=qkT_d.ap().rearrange("(c p) j -> p c j", p=128),
                )
                nc.scalar.dma_start(out=ones_r, in_=ones_d[:, :])
                nc.scalar.dma_start(out=bo_row, in_=bo_d[:, :])

            def emit_wo_dma():
                nc.scalar.dma_start(
                    out=woT4.rearrange("p (c e) -> p c e", c=EC),
                    in_=woT_d.ap().rearrange("(c p) e -> p c e", p=128),
                )

            # xT: [512 e, 2048 n] packed as [128, EC * NSH], bf16
            xTall = xta.tile([128, EC * NSH], BF16, tag="xT")
            xT3 = xTall.rearrange("p (c n) -> p c n", c=EC)

            # OT in SBUF: [128 e-in-chunk, (chunk|graph|q) 512], fp32r for
            # the output projection; filled by quadrant copies from psum
            ot_sb = finp.tile([128, 512], BF16, tag="otsb")

            def emit_transpose_block(g):
                # plain fp32 HWDGE loads (fast); cast to bf16 on ACT/DVE.
                # Graph 0 split in 4 chunks so the PE can start sooner.
                nparts = 4 if g == 0 else 1
                xns = []
                for part in range(nparts):
                    rows = 512 // nparts
                    xn = xnp.tile(
                        [128, (rows // 128) * E], F32, tag="xn", name=f"xn{g}_{part}"
                    )
                    src = xs_d.ap()[
                        g * 512 + part * rows : g * 512 + (part + 1) * rows, :
                    ].rearrange("(r p) e -> p r e", p=128)
                    nc.sync.dma_start(
                        out=xn.rearrange("p (r e) -> p r e", e=E), in_=src
                    )
                    xns.append(xn)
                for r in range(4):
                    nb = 4 * g + r
                    xn = xns[r] if nparts == 4 else xns[0]
                    roff = 0 if nparts == 4 else r * E
                    xnb = xnp.tile([128, E], BF16, tag="xnb", name="xnb")
                    if nb % 2 == 0:
                        nc.scalar.activation(
                            out=xnb,
                            in_=xn[:, roff : roff + E],
                            func=mybir.ActivationFunctionType.Copy,
                        )
                    else:
                        nc.vector.tensor_copy(xnb, xn[:, roff : roff + E])
                    roff = 0
                    xn = xnb
                    pt = pst.tile([128, 512], BF16, tag="tp", name="pt")
                    for c in range(EC):
                        nc.tensor.transpose(
                            pt[:, c * 128 : (c + 1) * 128],
                            xn[:, roff + c * 128 : roff + (c + 1) * 128],
                            identb,
                        )
                    # one strided copy: psum block c -> xT[c][:, nb*128:...]
                    # alternate ACT / DVE to balance engine load
                    if nb % 2 == 0:
                        nc.scalar.activation(
                            out=xT3[:, :, nb * 128 : (nb + 1) * 128],
                            in_=pt.rearrange("p (c n) -> p c n", c=EC),
                            func=mybir.ActivationFunctionType.Copy,
                        )
                    else:
                        nc.vector.tensor_copy(
                            xT3[:, :, nb * 128 : (nb + 1) * 128],
                            pt.rearrange("p (c n) -> p c n", c=EC),
                        )

            def emit_graph(g):
                ns = g * NODES_PER_GRAPH  # node offset in xT free dim

                # ---- V per node chunk: [128 n, 512 eout] (bf16) ----
                vtiles = []
                for c in range(EC):
                    vps = psw.tile([128, 512], F32, tag="work", name="vps")
                    for e in range(EC):
                        nc.tensor.matmul(
                            vps,
                            lhsT=xTall[
                                :, e * NSH + ns + c * 128 : e * NSH + ns + (c + 1) * 128
                            ],
                            rhs=wvTs[e],
                            start=(e == 0),
                            stop=(e == EC - 1),
                        )
                    vt = vtp.tile([128, 512], BF16, tag="vt", name="vt")
                    nc.vector.tensor_copy(vt, vps)
                    vtiles.append(vt)

                # ---- scores + softmax, per head-quad q (4 heads) ----
                # scores = (Qs @ wk) @ x.T directly from xT; the key bias
                # bk only shifts each row by a constant, which softmax
                # ignores, so it is dropped
                attnTs = []
                for q in range(2):
                    sps = psw.tile([128, 512], F32, tag="work", name="sps")
                    for c in range(EC):
                        nc.tensor.matmul(
                            sps,
                            lhsT=qkt[:, c * 256 + q * 128 : c * 256 + (q + 1) * 128],
                            rhs=xTall[:, c * NSH + ns : c * NSH + ns + 512],
                            start=(c == 0),
                            stop=(c == EC - 1),
                        )
                    expt = smp.tile([128, 512], F32, tag="exp", name="expt")
                    sumt = smp.tile([128, 1], F32, tag="sum", name="sumt")
                    nc.scalar.activation(
                        out=expt,
                        in_=sps,
                        func=mybir.ActivationFunctionType.Exp,
                        accum_out=sumt,
                    )
                    rect = smp.tile([128, 1], F32, tag="rec", name="rect")
                    nc.vector.reciprocal(out=rect, in_=sumt)
                    attnt = smp.tile([128, 512], BF16, tag="attn", name="attnt")
                    nc.vector.tensor_scalar_mul(attnt, expt, rect)

                    atps = pst.tile([128, 512], BF16, tag="tp", name="atps")
                    for c in range(EC):
                        nc.tensor.transpose(
                            atps[:, c * 128 : (c + 1) * 128],
                            attnt[:, c * 128 : (c + 1) * 128],
                            identb,
                        )
                    attnT = smp.tile([128, 512], BF16, tag="attnT", name="attnT")
                    nc.vector.tensor_copy(attnT, atps)
                    attnTs.append(attnT)

                # ---- AV quadrants: per head-pair ce ----
                # lhsT = V [128 n, 128 (2h d)], rhs = attnT two-head cols
                # [128 n, 64]; diagonal quadrants accumulate over chunks c
                for ce in range(EC):
                    avq = psq.tile([128, 64], F32, tag="avq", name="avq")
                    aT = attnTs[ce // 2]
                    hq0 = 2 * (ce % 2)  # first head's col block within quad
                    for c in range(EC):
                        nc.tensor.matmul(
                            avq,
                            lhsT=vtiles[c][:, ce * 128 : (ce + 1) * 128],
                            rhs=aT[:, c * 128 + hq0 * 32 : c * 128 + (hq0 + 2) * 32],
                            start=(c == 0),
                            stop=(c == EC - 1),
                        )
                    # extract diagonal quadrants into OT layout
                    for a in range(2):
                        dst = ot_sb[
                            a * 64 : (a + 1) * 64,
                            ce * 128 + g * 32 : ce * 128 + (g + 1) * 32,
                        ]
                        srcq = avq[a * 64 : (a + 1) * 64, a * 32 : (a + 1) * 32]
                        if (ce + a) % 2 == 0:
                            nc.scalar.activation(
                                out=dst,
                                in_=srcq,
                                func=mybir.ActivationFunctionType.Copy,
                            )
                        else:
                            nc.vector.tensor_copy(dst, srcq)

            def emit_outproj(g):
                # bf16 out-projection for graph g only: overlaps with the
                # next graph's compute instead of serializing at the end
                xcps = psx.tile([32, 512], F32, tag="xc", name="xcps")
                nc.tensor.matmul(
                    xcps, lhsT=ones_r[:, :32], rhs=bo_row, start=True, stop=False
                )
                for c in range(EC):
                    nc.tensor.matmul(
                        xcps,
                        lhsT=ot_sb[:, c * 128 + g * 32 : c * 128 + (g + 1) * 32],
                        rhs=woTs[c],
                        start=False,
                        stop=(c == EC - 1),
                    )
                xc_sb = finp.tile([32, 512], F32, tag="xcsb", name="xc_sb")
                if g % 2 == 0:
                    nc.scalar.activation(
                        out=xc_sb, in_=xcps, func=mybir.ActivationFunctionType.Copy
                    )
                else:
                    nc.vector.tensor_copy(xc_sb, xcps)
                nc.sync.dma_start(out=out_d[g * 32 : (g + 1) * 32, :], in_=xc_sb)

            emit_transpose_block(0)
            emit_const_dmas()
            for g in range(GPC):
                if g + 1 < GPC:
                    emit_transpose_block(g + 1)
                if g == 0:
                    emit_wo_dma()
                emit_graph(g)
                emit_outproj(g)

    nc.finalize()
    return nc


def _host_prep(x, xcent_base, in_proj_w, in_proj_b, out_proj_w, out_proj_b):
    x = np.asarray(x, np.float32)
    ipw = np.asarray(in_proj_w, np.float32)
    ipb = np.asarray(in_proj_b, np.float32)
    wo = np.asarray(out_proj_w, np.float32)
    bo = np.asarray(out_proj_b, np.float32)
    xcb = np.asarray(xcent_base, np.float32)

    wq, wk, wv = ipw[0:E], ipw[E : 2 * E], ipw[2 * E : 3 * E]
    bq, bk, bv = ipb[0:E], ipb[E : 2 * E], ipb[2 * E : 3 * E]

    # query projection (identical for every graph), folded with wk:
    # QK_h = Qs_h @ wk_h  ->  scores_h = QK_h @ x.T
    qs = (xcb @ wq.T + bq) * np.float32(1.0 / np.sqrt(HD))  # [32, 512]
    qkT = np.zeros((E, 256), np.float32)
    for h in range(H):
        qk_h = qs[:, h * HD : (h + 1) * HD] @ wk[h * HD : (h + 1) * HD, :]  # [32, E]
        q_, hq = divmod(h, 4)
        qkT[:, q_ * 128 + hq * 32 : q_ * 128 + (hq + 1) * 32] = qk_h.T

    bf = ml_dtypes.bfloat16
    consts = dict(
        wvT=np.ascontiguousarray(wv.T).astype(bf),
        woT=np.ascontiguousarray(wo.T).astype(bf),
        qkT=qkT.astype(bf),
        ones=np.ones((1, 128), bf),
        bo=np.ascontiguousarray((wo @ bv + bo)[None, :]).astype(bf),
    )
    return x, consts


def kernel(
    x,
    edge_index,
    batch,
    xcent_base,
    in_proj_w,
    in_proj_b,
    out_proj_w,
    out_proj_b,
):
    global LAST_RESULT
    x, consts = _host_prep(
        x, xcent_base, in_proj_w, in_proj_b, out_proj_w, out_proj_b
    )

    if "nc" not in _NC_CACHE:
        _NC_CACHE["nc"] = _build_nc()
    nc = _NC_CACHE["nc"]

    in_maps = [
        dict(consts, xs=np.ascontiguousarray(x[k * NSH : (k + 1) * NSH]))
        for k in range(NCORES)
    ]
    res = run_bass_kernel_spmd(nc, in_maps, list(range(NCORES)))
    LAST_RESULT = res

    xcent = np.concatenate(
        [res.results[k]["out"].reshape(GPC, RATIO, E) for k in range(NCORES)], axis=0
    ).astype(np.float32)

    batch = np.asarray(batch)
    batchcent = np.repeat(np.arange(B, dtype=batch.dtype), RATIO)
    return xcent, batchcent
